# revision 39
# baseline (speedup 1.0000x reference)
"""DeepFM (nn_DeepFM_77120432766994) Trainium2 Bass kernel.

Strategy: data-parallel over batch across 8 NeuronCores; per core 2048
batch rows in 4 tiles of 512.

Gather plan (the perf-critical part; SWDGE instructions cost ~1us fixed
each on the Pool engine, so instruction count dominates):
  - fields 2..9 (vocab 1008,1004,102,1000,500,50,112,107; 3883 rows
    total): ONE InstDMAGatherAnt per tile gathers all 8 fields x 512
    samples (4096 rows) from a concatenated subtable with 256B rows
    (64 f32: 32 emb + bias + zero pad) using int16 local indices.
  - fields 0,1 (vocab 1e6 / 1e5; indices exceed the gather's int16
    range): classic per-partition indirect DMA, 128 rows/instruction,
    8 instructions per tile, from a narrow [V, 33] f32 table.
  Total SWDGE instructions per tile: 9 (vs 40 for all-indirect).

Compute (as the prior all-indirect version):
  - staging tiles packed by DVE into G [128, 4, 330] f32 (sample-major,
    field f at cols 33f..33f+32 = emb dims + bias) so PE transposes see
    one source written exclusively by DVE (keeps every PE instruction
    at <= 1 new sync-wait).
  - 12 PE transposes -> X chunks [128feat, 512batch]; dense rows via
    host-transposed staging; K=90 partition slices exclude garbage.
  - FM: masked matmuls: S[e,b] (field sums; bias-sum in row 32),
    -0.5*sum(x^2) via squared X; 0.5*||s||^2 via ACT square + matmul.
  - MLP: matmul chains with fused bias+ReLU on the scalar engine; all
    accumulated into one [1,512] PSUM bank -> sigmoid -> y.
"""

import os
import sys

import numpy as np

for _p in ("/opt/trn_rl_repo", "/root/.axon_site/_ro/trn_rl_repo"):
    if os.path.isdir(_p) and _p not in sys.path:
        sys.path.insert(0, _p)

import concourse.bass as bass
import concourse.bacc as bacc
import concourse.mybir as mybir
import concourse.tile as tile
from concourse.bass_utils import run_bass_kernel_spmd

# ---------------------------------------------------------------- constants
FIELD_DIMS = [1000000, 100000, 1008, 1004, 102, 1000, 500, 50, 112, 107]
NF = 10
NSMALL = 8                    # fields 2..9 go through the bulk gather
NBIG = 2                      # fields 0,1 via indirect DMA
B = 16384
EMB = 32
DENSE = 16
MLP = [256, 128, 64]
BN_EPS = 1e-5
V = int(np.sum(FIELD_DIMS))
OFFSETS = np.concatenate([[0], np.cumsum(FIELD_DIMS)[:-1]]).astype(np.int64)
VSMALL = int(np.sum(FIELD_DIMS[2:]))          # 3883
SM_OFF = np.concatenate([[0], np.cumsum(FIELD_DIMS[2:])[:-1]]).astype(np.int64)

N_CORES = 8
B_LOC = B // N_CORES          # 2048
NT = 4                        # tiles per core
BT = 512                      # batch per tile
NS = 4                        # subtiles (of 128) per tile
P = 128

FW = 33                       # field width in feature layout (32 emb + bias)
GW = 64                       # gather row width (256B: FW used + zero pad)
D0 = NF * FW                  # 330 packed feature columns
NCH = 3                       # k-chunks: [0:128), [128:256), [256:346)
KC = [P, P, 112]              # contraction size per chunk (incl. dense rows)
DROW = 96                     # dense rows begin here within chunk 2 (32-aligned)

NSM_IDX = NS * P * NSMALL     # 4096 small-field rows gathered per tile
NGI = 1024                    # max num_idxs per gather (SWDGE ring capacity)
NGATH = NSM_IDX // NGI        # 4 gather instructions per tile

V1 = FIELD_DIMS[1]            # 100000
NW1 = 4                       # f1 windows of 32768 rows (int16-addressable)
W1SZ = 32768

F32 = mybir.dt.float32
F32R = mybir.dt.float32r
I32 = mybir.dt.int32
I16 = mybir.dt.int16

USE_F32R = True               # full-speed PE path; flip to False for exact fp32
MMDT = F32R if USE_F32R else F32

# wall (f32r weight blob) column offsets
W0_O = 0                      # 3 chunks x 256
W1_O = 768                    # 2 chunks x 128
W2_O = 1024                   # 64
WO_O = 1088                   # 1 (rows 0:64)
FM_O = 1089                   # 3 chunks x 34
HV_O = 1191                   # 1 (rows 0:32, value 0.5)
WALL_W = 1192
# aux (f32 blob) column offsets
ID_O = 0                      # identity 128
B1_O = 128                    # 2
B2_O = 130                    # 1
B3_O = 131                    # 1 (rows 0:64)
SC_O = 132                    # 1 (row 0: bo + fm_bias)
AUX_W = 133


# ---------------------------------------------------------------- device code
def _build_nc(reps=1):
    nc = bacc.Bacc("TRN2", target_bir_lowering=False, debug=False,
                   num_swdge_queues=4)

    tab = nc.dram_tensor("tab", [V, FW], F32, kind="ExternalInput").ap()
    tabs = nc.dram_tensor("tabs", [VSMALL, GW], F32, kind="ExternalInput").ap()
    tabs1 = nc.dram_tensor("tabs1", [V1, GW], F32, kind="ExternalInput").ap()
    gidx = nc.dram_tensor("gidx", [NT, P, NS], I32,
                          kind="ExternalInput").ap()
    f1idx = nc.dram_tensor("f1idx", [NT, P, NW1 * (BT // 16)], I16,
                           kind="ExternalInput").ap()
    f1msk = nc.dram_tensor("f1msk", [NT, P, NW1 * NS], F32,
                           kind="ExternalInput").ap()
    sidx = nc.dram_tensor("sidx", [NT, P, NGATH * (NGI // 16)], I16,
                          kind="ExternalInput").ap()
    denset = nc.dram_tensor("denset", [NT, DENSE, BT], F32,
                            kind="ExternalInput").ap()
    walld = nc.dram_tensor("walld", [P, WALL_W], MMDT, kind="ExternalInput").ap()
    auxd = nc.dram_tensor("auxd", [P, AUX_W], F32, kind="ExternalInput").ap()
    y = nc.dram_tensor("y", [NT, BT], F32, kind="ExternalOutput").ap()

    from contextlib import ExitStack
    with tile.TileContext(nc) as tc, ExitStack() as ctx:
        wp = ctx.enter_context(tc.tile_pool(name="weights", bufs=1))

        wall = wp.tile([P, WALL_W], MMDT, tag="wall")
        nc.sync.dma_start(out=wall[:], in_=walld[:])
        aux = wp.tile([P, AUX_W], F32, tag="aux")
        nc.sync.dma_start(out=aux[:], in_=auxd[:])
        dummy = wp.tile([1, 1], F32, tag="dummy")

        ident = aux[:, ID_O:ID_O + P]

        def w0(c, o):
            kc = KC[c]
            return wall[0:kc, W0_O + c * MLP[0] + o * P:W0_O + c * MLP[0] + (o + 1) * P]

        def w1(k):
            return wall[:, W1_O + k * MLP[1]:W1_O + (k + 1) * MLP[1]]

        def fmw_s(c):
            return wall[0:KC[c], FM_O + c * 34:FM_O + c * 34 + FW]

        def fmw_q(c):
            return wall[0:KC[c], FM_O + c * 34 + FW:FM_O + c * 34 + 34]

        ip = ctx.enter_context(tc.tile_pool(name="idx", bufs=2))
        sip = ctx.enter_context(tc.tile_pool(name="sidxp", bufs=2))
        gsp = ctx.enter_context(tc.tile_pool(name="gsmall", bufs=2))
        gfp = ctx.enter_context(tc.tile_pool(name="gf01", bufs=2))
        s1p = ctx.enter_context(tc.tile_pool(name="s1idx", bufs=2))
        m1p = ctx.enter_context(tc.tile_pool(name="m1", bufs=2))
        g1p = ctx.enter_context(tc.tile_pool(name="g1", bufs=2))
        g1tp = ctx.enter_context(tc.tile_pool(name="g1tmp", bufs=8))
        gp = ctx.enter_context(tc.tile_pool(name="gpack", bufs=2))
        dsp = ctx.enter_context(tc.tile_pool(name="dstage", bufs=2))
        xp = ctx.enter_context(tc.tile_pool(name="xchunks", bufs=6))
        xqp = ctx.enter_context(tc.tile_pool(name="xsq", bufs=2))
        s2p = ctx.enter_context(tc.tile_pool(name="s2", bufs=2))
        hp = ctx.enter_context(tc.tile_pool(name="acts", bufs=6))
        yp = ctx.enter_context(tc.tile_pool(name="yout", bufs=2))

        ps_x = ctx.enter_context(tc.tile_pool(name="ps_x", bufs=2, space="PSUM"))
        ps_s = ctx.enter_context(tc.tile_pool(name="ps_s", bufs=1, space="PSUM"))
        ps_fm = ctx.enter_context(tc.tile_pool(name="ps_fm", bufs=1, space="PSUM"))
        ps_h1 = ctx.enter_context(tc.tile_pool(name="ps_h1", bufs=2, space="PSUM"))
        ps_h2 = ctx.enter_context(tc.tile_pool(name="ps_h2", bufs=1, space="PSUM"))
        ps_h3 = ctx.enter_context(tc.tile_pool(name="ps_h3", bufs=1, space="PSUM"))

        # Absorber instructions: make PE/ACT observe the weight/aux DMA
        # semaphores via single-wait instructions before any real consumer.
        xps_a = ps_x.tile([P, BT], F32, tag="xps")
        nc.tensor.transpose(out=xps_a[:, 0:P], in_=ident, identity=ident)
        nc.tensor.transpose(out=xps_a[:, P:2 * P], in_=wall[:, 0:P].bitcast(F32),
                            identity=ident)
        nc.scalar.copy(dummy[:], aux[0:1, B1_O:B1_O + 1])

        import contextlib
        loop_cm = tc.For_i(0, reps, 1) if reps > 1 else contextlib.nullcontext()
        with loop_cm:
          for t in range(NT):
            # ---- small fields: 4 gathers of 1024 rows (ring cap), one per
            # subtile: gather g covers sample sub-block g's 8 fields.
            si = sip.tile([P, NGATH * (NGI // 16)], I16, tag="si")
            nc.sync.dma_start(out=si[:], in_=sidx[t])
            Gs = gsp.tile([P, (NSM_IDX // P) * GW], F32, tag="Gs")
            Gsv = Gs[:].rearrange("p (c e) -> p c e", c=NSM_IDX // P)
            W16 = NGI // 16
            for g in range(NGATH):
                nc.gpsimd.dma_gather(
                    out_ap=Gsv[:, g * (NGI // P):(g + 1) * (NGI // P), :],
                    in_ap=tabs,
                    idxs_ap=si[:, g * W16:(g + 1) * W16],
                    num_idxs=NGI,
                    num_idxs_reg=NGI,
                    elem_size=GW,
                    transpose=False,
                    queue_num=g,
                )

            # ---- field 0: per-partition indirect DMA (4 insts)
            idx = ip.tile([P, NS], I32, tag="idx")
            nc.sync.dma_start(out=idx[:], in_=gidx[t])
            Gf = gfp.tile([P, NS * FW], F32, tag="Gf")
            Gf3 = Gf[:].rearrange("p (s f) -> p s f", s=NS)
            for s in range(NS):
                bi = nc.gpsimd.indirect_dma_start(
                    out=Gf3[:, s, :],
                    out_offset=None,
                    in_=tab,
                    in_offset=bass.IndirectOffsetOnAxis(
                        ap=idx[:, s:s + 1], axis=0),
                )
                if s % 4:
                    bi.ins.queue = "qPoolDynamic%d" % (s % 4)

            # ---- field 1: 4 windowed dma_gathers (all 512 samples each,
            # dummy idx 0 when out-of-window) + mask-select merge on DVE.
            si1 = s1p.tile([P, NW1 * (BT // 16)], I16, tag="si1")
            nc.sync.dma_start(out=si1[:], in_=f1idx[t])
            mk1 = m1p.tile([P, NW1 * NS], F32, tag="mk1")
            nc.sync.dma_start(out=mk1[:], in_=f1msk[t])
            G1 = g1p.tile([P, NW1 * NS * GW], F32, tag="G1")
            G1v = G1[:].rearrange("p (w s e) -> p w s e", w=NW1, s=NS)
            W16B = BT // 16
            for w in range(NW1):
                lo = w * W1SZ
                hi = min(V1, lo + W1SZ)
                nc.gpsimd.dma_gather(
                    out_ap=G1v[:, w, :, :],
                    in_ap=tabs1[lo:hi],
                    idxs_ap=si1[:, w * W16B:(w + 1) * W16B],
                    num_idxs=BT,
                    num_idxs_reg=BT,
                    elem_size=GW,
                    transpose=False,
                    queue_num=w,
                )

            dstage = dsp.tile([DENSE, BT], F32, tag="dstage")
            nc.sync.dma_start(out=dstage[:], in_=denset[t])

            # ---- DVE packs -> G [128, NS, 330] (written exclusively by DVE)
            G = gp.tile([P, NS * D0], F32, tag="G")
            G3 = G[:].rearrange("p (s f) -> p s f", s=NS)
            # small rows: Gs[p, (s*8+fi), 0:33] -> G3[p, s, (2+fi)*33 ...]
            Gs4 = Gs[:].rearrange("p (s fi e) -> p s fi e", s=NS, fi=NSMALL)
            G4 = G[:].rearrange("p (s fi e) -> p s fi e", s=NS, fi=NF)
            nc.vector.tensor_copy(G4[:, :, NBIG:NF, 0:FW], Gs4[:, :, :, 0:FW])
            # f0 rows
            Gf4 = Gf[:].rearrange("p (s fi e) -> p s fi e", s=NS, fi=1)
            nc.vector.tensor_copy(G4[:, :, 0:1, :], Gf4[:])
            # f1 rows: sum_w G1[w] * mask[w]
            mk3 = mk1[:].rearrange("p (w s) -> p w s", w=NW1)
            g4f1 = G4[:, :, 1, 0:FW]
            nc.vector.tensor_tensor(
                out=g4f1, in0=G1v[:, 0, :, 0:FW],
                in1=mk3[:, 0, :, None].to_broadcast([P, NS, FW]),
                op=mybir.AluOpType.mult)
            for w in range(1, NW1):
                tmp = g1tp.tile([P, NS * FW], F32, tag="g1t")
                tmp3 = tmp[:].rearrange("p (s e) -> p s e", s=NS)
                nc.vector.tensor_tensor(
                    out=tmp3[:], in0=G1v[:, w, :, 0:FW],
                    in1=mk3[:, w, :, None].to_broadcast([P, NS, FW]),
                    op=mybir.AluOpType.mult)
                nc.vector.tensor_add(g4f1, g4f1, tmp3[:])

            # ---- transpose to feature-major chunks
            Xs = []
            for c in range(NCH):
                w = min(P, D0 - c * P)      # 128 / 128 / 74 gathered cols
                xps = ps_x.tile([P, BT], F32, tag="xps")
                for s in range(NS):
                    nc.tensor.transpose(
                        out=xps[0:w, s * P:(s + 1) * P],
                        in_=G3[:, s, c * P:c * P + w],
                        identity=ident,
                    )
                X = xp.tile([P, BT], MMDT, tag="X")
                if c == 2:
                    # zero the 64:96 band so the K=112 matmul reads no garbage
                    nc.vector.tensor_copy(
                        X[64:DROW, :],
                        wall[64:DROW, HV_O:HV_O + 1].to_broadcast([DROW - 64, BT]))
                nc.vector.tensor_copy(X[0:w, :], xps[0:w, :])
                if c == 2:
                    nc.vector.tensor_copy(X[DROW:DROW + DENSE, :], dstage[:])
                Xs.append(X)

            # ---- FM: S rows 0..31 = per-dim field sums, row 32 = bias sum
            sp = ps_s.tile([FW, BT], F32, tag="sp")
            for c in range(NCH):
                nc.tensor.matmul(sp[:], lhsT=fmw_s(c), rhs=Xs[c][0:KC[c], :],
                                 start=(c == 0), stop=(c == NCH - 1))

            fmp = ps_fm.tile([1, BT], F32, tag="fmp")
            for c in range(NCH):
                kc = KC[c]
                Xq = xqp.tile([P, BT], MMDT, tag="Xq")
                nc.vector.tensor_mul(Xq[0:kc, :], Xs[c][0:kc, :], Xs[c][0:kc, :])
                nc.tensor.matmul(fmp[:], lhsT=fmw_q(c), rhs=Xq[0:kc, :],
                                 start=(c == 0), stop=False)
            S2 = s2p.tile([EMB, BT], MMDT, tag="S2")
            nc.scalar.square(S2[:], sp[0:EMB, :])
            nc.tensor.matmul(fmp[:], lhsT=wall[0:EMB, HV_O:HV_O + 1], rhs=S2[:],
                             start=False, stop=False)

            # ---- MLP layer 0: h1[o, b], o in 2 chunks of 128
            h1s = []
            for o in range(2):
                h1p = ps_h1.tile([P, BT], F32, tag="h1p")
                for c in range(NCH):
                    nc.tensor.matmul(h1p[:], lhsT=w0(c, o), rhs=Xs[c][0:KC[c], :],
                                     start=(c == 0), stop=(c == NCH - 1))
                h1 = hp.tile([P, BT], MMDT, tag="h1")
                nc.scalar.activation(h1[:], h1p[:],
                                     mybir.ActivationFunctionType.Relu,
                                     bias=aux[:, B1_O + o:B1_O + o + 1])
                h1s.append(h1)

            # layer 1
            h2p = ps_h2.tile([P, BT], F32, tag="h2p")
            for k in range(2):
                nc.tensor.matmul(h2p[:], lhsT=w1(k), rhs=h1s[k][:],
                                 start=(k == 0), stop=(k == 1))
            h2 = hp.tile([P, BT], MMDT, tag="h2")
            nc.scalar.activation(h2[:], h2p[:],
                                 mybir.ActivationFunctionType.Relu,
                                 bias=aux[:, B2_O:B2_O + 1])

            # layer 2
            h3p = ps_h3.tile([MLP[2], BT], F32, tag="h3p")
            nc.tensor.matmul(h3p[:], lhsT=wall[:, W2_O:W2_O + MLP[2]], rhs=h2[:],
                             start=True, stop=True)
            h3 = hp.tile([MLP[2], BT], MMDT, tag="h3")
            nc.scalar.activation(h3[:], h3p[:],
                                 mybir.ActivationFunctionType.Relu,
                                 bias=aux[0:MLP[2], B3_O:B3_O + 1])

            # output layer into the FM accumulator
            nc.tensor.matmul(fmp[:], lhsT=wall[0:MLP[2], WO_O:WO_O + 1], rhs=h3[:],
                             start=False, stop=True)

            # presig = fmp + bias_sum row; y = sigmoid(presig + (bo + fm_bias))
            bsum = yp.tile([1, BT], F32, tag="bsum")
            nc.vector.tensor_copy(bsum[:], sp[EMB:FW, :])
            pres = yp.tile([1, BT], F32, tag="pres")
            nc.vector.tensor_add(pres[:], fmp[:], bsum[:])
            ysb = yp.tile([1, BT], F32, tag="ysb")
            nc.scalar.activation(ysb[:], pres[:],
                                 mybir.ActivationFunctionType.Sigmoid,
                                 bias=aux[0:1, SC_O:SC_O + 1])
            nc.sync.dma_start(out=y[t:t + 1, :], in_=ysb[:])

    nc.compile()
    return nc


_NC = None


def _get_nc():
    global _NC
    if _NC is None:
        _NC = _build_nc()
    return _NC


# ---------------------------------------------------------------- host prep
def _prep_shared(emb_table, bias_table, fm_bias, Wo, bo,
                 W0, b0, g0, be0, W1, b1, g1, be1, W2, b2, g2, be2):
    inv = np.float32(1.0 / np.sqrt(1.0 + BN_EPS))

    tab = np.empty([V, FW], np.float32)
    tab[:, :EMB] = emb_table
    tab[:, EMB] = bias_table[:, 0]

    # small-field subtable with 256B rows (zero padded)
    tabs = np.zeros([VSMALL, GW], np.float32)
    lo = int(OFFSETS[2])
    tabs[:, :FW] = tab[lo:lo + VSMALL]

    # f1 subtable with 256B rows for the windowed gathers
    tabs1 = np.zeros([V1, GW], np.float32)
    lo1 = int(OFFSETS[1])
    tabs1[:, :FW] = tab[lo1:lo1 + V1]

    def fold(Wl, bl, gl, bel):
        s = (gl * inv).astype(np.float32)
        return (Wl * s[:, None]).astype(np.float32), (bl * s + bel).astype(np.float32)

    W0f, b0f = fold(W0, b0, g0, be0)
    W1f, b1f = fold(W1, b1, g1, be1)
    W2f, b2f = fold(W2, b2, g2, be2)

    # feature permutation: model col 32f+e -> layout row 33f+e; dense -> 330+d
    w0t = np.zeros([NCH * P, MLP[0]], np.float32)
    for f in range(NF):
        w0t[f * FW:f * FW + EMB, :] = W0f[:, f * EMB:(f + 1) * EMB].T
    w0t[2 * P + DROW:2 * P + DROW + DENSE, :] = W0f[:, NF * EMB:].T

    fmw = np.zeros([NCH * P, 34], np.float32)
    for f in range(NF):
        for e in range(EMB):
            fmw[f * FW + e, e] = 1.0       # field-sum matrix
            fmw[f * FW + e, 33] = -0.5     # -0.5 * sum-of-squares mask
        fmw[f * FW + EMB, 32] = 1.0        # bias-sum mask

    wall = np.zeros([P, WALL_W], np.float32)
    for c in range(NCH):
        wall[:, W0_O + c * MLP[0]:W0_O + (c + 1) * MLP[0]] = w0t[c * P:(c + 1) * P]
    for k in range(2):
        wall[:, W1_O + k * MLP[1]:W1_O + (k + 1) * MLP[1]] = \
            W1f.T[k * P:(k + 1) * P]
    wall[:, W2_O:W2_O + MLP[2]] = W2f.T
    wall[0:MLP[2], WO_O] = Wo[0].astype(np.float32)
    for c in range(NCH):
        wall[:, FM_O + c * 34:FM_O + (c + 1) * 34] = fmw[c * P:(c + 1) * P]
    wall[0:EMB, HV_O] = 0.5

    auxa = np.zeros([P, AUX_W], np.float32)
    auxa[:, ID_O:ID_O + P] = np.eye(P, dtype=np.float32)
    for o in range(2):
        auxa[:, B1_O + o] = b0f[o * P:(o + 1) * P]
    auxa[:, B2_O] = b1f
    auxa[0:MLP[2], B3_O] = b2f
    auxa[0, SC_O] = np.float32(bo[0]) + np.float32(fm_bias[0])

    return dict(tab=tab, tabs=tabs, tabs1=tabs1, walld=wall, auxd=auxa)


def _pack_small_idx(sp_loc):
    """sp_loc: [B_LOC, NSMALL] int64 local small-table rows ->
    [NT, NGATH, 128, NGI//16] int16 in dma_gather's wrapped layout.
    Gather g of tile t covers subtile g: item j = fi*128+p lands at
    out[p, chunk g*8+fi] matching sample g*128+p field fi; idx j lives
    at partition j%16, col j//16, replicated across the 8 groups of 16
    partitions."""
    out = np.zeros([NT, NGATH, P, NGI // 16], np.int16)
    v = sp_loc.reshape(NT, NS, P, NSMALL)               # [t, g, p, fi]
    iv = v.transpose(0, 1, 3, 2).reshape(NT, NGATH, NGI)  # j = fi*128+p
    jj = np.arange(NGI)
    for t in range(NT):
        for g in range(NGATH):
            blk = np.zeros([16, NGI // 16], np.int16)
            blk[jj % 16, jj // 16] = iv[t, g].astype(np.int16)
            out[t, g] = np.tile(blk, (8, 1))
    # device tile holds gathers side by side: [t, p, g*(NGI//16)+w]
    return np.ascontiguousarray(out.transpose(0, 2, 1, 3).reshape(
        NT, P, NGATH * (NGI // 16)))


def _pack_f1(f1_loc):
    """f1_loc: [B_LOC] int64 local f1 rows -> (idx [NT,P,NW1*32] i16 in
    dma_gather's wrapped layout with dummy 0 when out-of-window,
    msk [NT,P,NW1*NS] f32 window-membership masks)."""
    idx_out = np.zeros([NT, NW1, P, BT // 16], np.int16)
    msk_out = np.zeros([NT, NW1, P, NS], np.float32)
    v = f1_loc.reshape(NT, BT)                          # j = s*128+p order
    jj = np.arange(BT)
    for t in range(NT):
        flat = v[t]
        for w in range(NW1):
            lo = w * W1SZ
            inw = (flat >= lo) & (flat < min(V1, lo + W1SZ))
            wi = np.where(inw, flat - lo, 0).astype(np.int16)
            blk = np.zeros([16, BT // 16], np.int16)
            blk[jj % 16, jj // 16] = wi
            idx_out[t, w] = np.tile(blk, (8, 1))
            msk_out[t, w] = inw.reshape(NS, P).T.astype(np.float32)
    return (np.ascontiguousarray(idx_out.transpose(0, 2, 1, 3).reshape(
                NT, P, NW1 * (BT // 16))),
            np.ascontiguousarray(msk_out.transpose(0, 2, 1, 3).reshape(
                NT, P, NW1 * NS)))


def _core_inputs(gl0, f1_loc, sp_loc, dense_inputs, c):
    lo = c * B_LOC
    g0 = gl0[lo:lo + B_LOC]                            # [2048] f0 global rows
    gidx = (g0.reshape(NT, NS, P)
            .transpose(0, 2, 1))                       # [NT, 128, NS]
    f1i, f1m = _pack_f1(f1_loc[lo:lo + B_LOC])
    sidx = _pack_small_idx(sp_loc[lo:lo + B_LOC])
    dt_ = (dense_inputs[lo:lo + B_LOC]
           .reshape(NT, BT, DENSE)
           .transpose(0, 2, 1))                        # [NT, DENSE, BT]
    return (np.ascontiguousarray(gidx), f1i, f1m, sidx,
            np.ascontiguousarray(dt_))


def kernel(sparse_inputs, dense_inputs, emb_table, bias_table, fm_bias,
           Wo, bo, W0, b0, g0, be0, W1, b1, g1, be1, W2, b2, g2, be2):
    sparse_inputs = np.asarray(sparse_inputs)
    dense_inputs = np.asarray(dense_inputs, dtype=np.float32)
    args = [np.asarray(a, dtype=np.float32) for a in
            (emb_table, bias_table, fm_bias, Wo, bo,
             W0, b0, g0, be0, W1, b1, g1, be1, W2, b2, g2, be2)]
    shared = _prep_shared(*args)

    sp = sparse_inputs.astype(np.int64)
    gl0 = (sp[:, 0] + OFFSETS[0]).astype(np.int32)                 # [B] f0
    f1_loc = sp[:, 1]                                              # [B] f1 local
    sp_loc = sp[:, NBIG:] + SM_OFF[None, :]                        # [B, 8] local

    in_maps = []
    for c in range(N_CORES):
        gidx, f1i, f1m, sidx, dt_ = _core_inputs(gl0, f1_loc, sp_loc,
                                                 dense_inputs, c)
        in_maps.append(dict(shared, gidx=gidx, f1idx=f1i, f1msk=f1m,
                            sidx=sidx, denset=dt_))

    nc = _get_nc()
    res = run_bass_kernel_spmd(nc, in_maps, list(range(N_CORES)),
                               trace=bool(os.environ.get("BASS_TRACE")))
    kernel.last_results = res

    out = np.empty([B], np.float32)
    for c in range(N_CORES):
        out[c * B_LOC:(c + 1) * B_LOC] = res.results[c]["y"].reshape(-1)
    return out

